# revision 47
# baseline (speedup 1.0000x reference)
"""GNN message-passing block on 8 Trainium2 NeuronCores.

Strategy (c-sharded, gather-free, fp8 streams):
- Shard pairs by center det (each det owns 32 consecutive pairs; 6250 dets/core).
- The neighbor gather f1[nIdxs] is eliminated: the host expands
  detFeatures[nIdxs] into a dense fp8 stream (pure data movement), and the
  device computes f1[n] = relu(W1^T detFn + b1) per pair as an extra K=128
  fp8 matmul.
- Layer 1 (z1 = Wp^T pairF + Wn^T f1n + Wc^T f1c):
  pair and neighbor terms are fused into ONE DoubleRow fp8 matmul per
  512-pair chunk (2 k-tiles of 32 at 0.5 cycles/row); the pairF stream is
  DMA'd and the f1n relu is written into adjacent slots of one SBUF tile so
  the DR moving AP covers both.  The center term stays an fp16 broadcast
  matmul; bias bp0 stays exact in the h1 activation.
- z2 wave uses 4 distinct PE tile positions (one concurrent wave).
- Segment max (32 consecutive pairs) is split between DVE and GPSIMD(Pool).
- Residual + output streams in bf16; phase-3 head is interleaved into the
  main loop every 16 supertiles to hide its tail.
"""

import sys

sys.path.insert(0, "/opt/trn_rl_repo")

import numpy as np

import concourse.bass as bass
import concourse.tile as tile
from concourse import bacc, mybir
from concourse.bass_utils import run_bass_kernel_spmd

F8 = mybir.dt.float8e4
F16 = mybir.dt.float16
BF16 = mybir.dt.bfloat16
F32 = mybir.dt.float32

N_DETS = 50000
KN = 32
N_CORES = 8
DC_REAL = N_DETS // N_CORES          # 6250 real dets per core
DC = 6272                            # padded dets per core (98 * 64)
S = DC // 64                         # 98 supertiles (64 dets / 2048 pairs each)
PAIRS = DC * KN                      # 200704 padded pairs per core
F1C = S * 16                         # 1568 cols of f1packed
PC = S * 32                          # 3136 pooled cols
PCP = 3584                           # pooled cols padded to 7*512
PT3 = PCP // 512                     # 7 phase-3 tiles
GRP = 4                              # supertiles per DMA group
NG = (S + GRP - 1) // GRP            # 25 groups (last has 2 supertiles)
FSP = 384                            # f1n-relu cols on ACT (rest on DVE)
HH = [0, 1, 1, 0]                    # z1 partition-half of chunk q
AX = mybir.AxisListType.X
RELU = mybir.ActivationFunctionType.Relu
DR = mybir.MatmulPerfMode.DoubleRow
ADD = mybir.AluOpType.add
MAX = mybir.AluOpType.max

_CACHE = {}


def _build():
    nc = bacc.Bacc("TRN2", target_bir_lowering=False, debug=False)

    detft16 = nc.dram_tensor("detft16", [128, DC], F16, kind="ExternalInput")
    p8 = nc.dram_tensor("p8", [128, PAIRS // 4], F8, kind="ExternalInput")
    dfn8 = nc.dram_tensor("dfn8", [128, PAIRS], F8, kind="ExternalInput")
    resid_bf = nc.dram_tensor("resid_bf", [128, 2 * PCP], BF16, kind="ExternalInput")
    w1 = nc.dram_tensor("w1", [128, 32], F16, kind="ExternalInput")
    w1_8 = nc.dram_tensor("w1_8", [128, 32], F8, kind="ExternalInput")
    wp4_8 = nc.dram_tensor("wp4_8", [128, 64], F8, kind="ExternalInput")
    wn4 = nc.dram_tensor("wn4", [128, 64], F16, kind="ExternalInput")
    wc4 = nc.dram_tensor("wc4", [128, 64], F16, kind="ExternalInput")
    wp1_2 = nc.dram_tensor("wp1_2", [128, 64], F16, kind="ExternalInput")
    wq0_2 = nc.dram_tensor("wq0_2", [128, 64], F16, kind="ExternalInput")
    wq1_2 = nc.dram_tensor("wq1_2", [128, 64], F16, kind="ExternalInput")
    wo2 = nc.dram_tensor("wo2", [128, 128], F16, kind="ExternalInput")
    b1x4 = nc.dram_tensor("b1x4", [128, 1], F32, kind="ExternalInput")
    bp0x2 = nc.dram_tensor("bp0x2", [128, 1], F32, kind="ExternalInput")
    bp1x2 = nc.dram_tensor("bp1x2", [128, 1], F32, kind="ExternalInput")
    bq0x2 = nc.dram_tensor("bq0x2", [128, 1], F32, kind="ExternalInput")
    bq1x2 = nc.dram_tensor("bq1x2", [128, 1], F32, kind="ExternalInput")
    out_t = nc.dram_tensor("out_t", [128, 2 * PCP], BF16, kind="ExternalOutput")

    with tile.TileContext(nc) as tc:
        with tc.tile_pool(name="persist", bufs=1) as pp, \
             tc.tile_pool(name="dfn", bufs=2) as dfn_p, \
             tc.tile_pool(name="p8g", bufs=2) as p8_p, \
             tc.tile_pool(name="f1nb", bufs=3) as f1n_p, \
             tc.tile_pool(name="hbuf", bufs=3) as h_p, \
             tc.tile_pool(name="ph3", bufs=2) as ph3_p, \
             tc.tile_pool(name="psy", bufs=2, space="PSUM") as psy, \
             tc.tile_pool(name="psz", bufs=2, space="PSUM") as psz, \
             tc.tile_pool(name="psz2", bufs=1, space="PSUM") as psz2:

            # --- load weights / biases, split across both HWDGE engines and
            # ordered so the first iterations' dependencies land first
            def _ld(eng, name, dram, shape, dt):
                t = pp.tile(shape, dt, tag=name, name=name)
                eng.dma_start(t[:], dram[:])
                return t
            w1_t = _ld(nc.sync, "w1", w1, [128, 32], F16)[:]
            w18_t = _ld(nc.scalar, "w18", w1_8, [128, 32], F8)[:]
            wp8_t = _ld(nc.sync, "wp8", wp4_8, [128, 64], F8)[:]
            b1_t = _ld(nc.scalar, "b1", b1x4, [128, 1], F32)[:]
            wn4_t = _ld(nc.scalar, "wn4", wn4, [128, 64], F16)[:]
            wc4_t = _ld(nc.scalar, "wc4", wc4, [128, 64], F16)[:]
            bp0_t = _ld(nc.scalar, "bp0", bp0x2, [128, 1], F32)[:]
            wp1_t = _ld(nc.scalar, "wp1", wp1_2, [128, 64], F16)[:]

            # split detft load so phase-1 chunk 0 starts early
            detft_t = pp.tile([128, DC], F16)
            nc.sync.dma_start(detft_t[:, 0:2048], detft16[:, 0:2048])
            nc.sync.dma_start(detft_t[:, 2048:DC], detft16[:, 2048:DC])

            # phase-3 weights (not needed until supertile 16)
            wq0_t = _ld(nc.scalar, "wq0", wq0_2, [128, 64], F16)[:]
            wq1_t = _ld(nc.scalar, "wq1", wq1_2, [128, 64], F16)[:]
            wo_t = _ld(nc.scalar, "wo", wo2, [128, 128], F16)[:]
            bp1_t = _ld(nc.scalar, "bp1", bp1x2, [128, 1], F32)[:]
            bq0_t = _ld(nc.scalar, "bq0", bq0x2, [128, 1], F32)[:]
            bq1_t = _ld(nc.scalar, "bq1", bq1x2, [128, 1], F32)[:]

            # --- phase 1: f1packed[32q+f, 16s+i] = relu(W1^T detF[64s+16q+i] + b1)
            # chunk 0 runs up front; chunks 1-3 are interleaved into the main
            # loop (chunk c is only needed from supertile 32c onwards).
            f1pk = pp.tile([128, F1C], F16)
            P1CHUNKS = [(0, 512), (512, 512), (1024, 512), (1536, 32)]

            def emit_phase1(ci):
                c0, cn = P1CHUNKS[ci]
                ps1 = psy.tile([128, 512], F32, tag="ps1")
                ns = cn // 16
                s0 = c0 // 16
                dview = detft_t[:].rearrange("p (s g) -> p s g", g=64)
                for q in range(4):
                    rhs = dview[:, s0:s0 + ns, 16 * q:16 * q + 16]
                    nc.tensor.matmul(ps1[32 * q:32 * q + 32, :cn], w1_t, rhs,
                                     start=True, stop=True, tile_position=(0, 32 * q))
                nc.scalar.activation(f1pk[:, c0:c0 + cn], ps1[:, :cn], RELU,
                                     bias=b1_t, scale=1.0)

            emit_phase1(0)

            pooled_raw = pp.tile([128, PC], F16)
            pooled = pp.tile([128, PCP], F16)
            nc.vector.memset(pooled[:, PC:PCP], 0.0)

            # --- phase 2 machinery.  PE waves are emitted so that ADJACENT
            # waves never share a PSUM accumulation chain: the chain's drain
            # latency hides behind an independent wave, the PE stays busy,
            # and the p-state can ramp to the full 2.4 GHz clock.

            def issue_group(gi):
                """DMA the dfn8 + p8 streams for supertile group gi."""
                gsz = min(GRP, S - GRP * gi)
                d_t = dfn_p.tile([128, 2048 * GRP], F8, tag="dfn")
                nc.sync.dma_start(d_t[:, :2048 * gsz],
                                  dfn8[:, 2048 * GRP * gi:2048 * (GRP * gi + gsz)])
                p_t = p8_p.tile([128, 512 * GRP], F8, tag="p8g")
                nc.sync.dma_start(p_t[:, :512 * gsz],
                                  p8[:, 512 * GRP * gi:512 * (GRP * gi + gsz)])
                return d_t, p_t

            def emit_f1n_wave(s, d_t):
                g = s % GRP
                ps_y = psy.tile([128, 512], F32, tag="ps1", name=f"psy_{s}")
                for q in range(4):
                    nc.tensor.matmul(ps_y[32 * q:32 * q + 32, :], w18_t,
                                     d_t[:, 2048 * g + 512 * q:2048 * g + 512 * (q + 1)],
                                     start=True, stop=True, tile_position=(0, 32 * q))
                return ps_y

            def emit_f1n_relu(ps_y):
                """relu+bias of f1n -> fp16 tile (ACT)."""
                f1n = f1n_p.tile([128, 512], F16, tag="f1n")
                nc.scalar.activation(f1n[:], ps_y[:], RELU, bias=b1_t, scale=1.0)
                return f1n

            # chunk q's z1 half: [0,1,1,0] so the four z2 matmuls later get
            # four DISTINCT legal (row, col=out-partition) PE tiles
            def z1_slice(z1, q):
                hh = 64 * HH[q]
                return z1[hh:hh + 64, 512 * (q // 2):512 * (q // 2) + 512], (32 * q, hh)

            def emit_p_wave(s, p_t):
                g = s % GRP
                z1 = psz.tile([128, 1024], F32, tag="z1", name=f"z1_{s}")
                for q in range(4):
                    o, tp = z1_slice(z1, q)
                    nc.tensor.matmul(o, wp8_t[32 * q:32 * q + 32, :],
                                     p_t[32 * q:32 * q + 32, 512 * g:512 * g + 512],
                                     start=True, stop=False, tile_position=tp,
                                     skip_group_check=True)
                return z1

            def emit_n_wave(z1, f1n):
                for q in range(4):
                    o, tp = z1_slice(z1, q)
                    nc.tensor.matmul(o, wn4_t[32 * q:32 * q + 32, :],
                                     f1n[32 * q:32 * q + 32, :],
                                     start=False, stop=False, tile_position=tp,
                                     skip_group_check=True)

            def emit_c_wave(s, z1):
                for q in range(4):
                    o, tp = z1_slice(z1, q)
                    rhs = f1pk[32 * q:32 * q + 32, 16 * s:16 * s + 16].rearrange(
                        "p (d one) -> p d one", one=1).to_broadcast([32, 16, 32])
                    nc.tensor.matmul(o, wc4_t[32 * q:32 * q + 32, :], rhs,
                                     start=False, stop=True, tile_position=tp,
                                     skip_group_check=True)

            def emit_h1(z1):
                """h1 = relu(z1+bp0), halves in parallel on ACT and DVE so the
                z2 wave (which gates the loop cycle) isn't held by a full
                1.1us activation.  Two tiles -- same-tile writers serialize."""
                h1a = h_p.tile([128, 512], F16, tag="h1a")
                nc.scalar.activation(h1a[:], z1[:, 0:512], RELU,
                                     bias=bp0_t, scale=1.0)
                h1b = h_p.tile([128, 512], F16, tag="h1b")
                nc.vector.tensor_scalar(h1b[:], z1[:, 512:1024],
                                        bp0_t, 0.0, op0=ADD, op1=MAX)
                return (h1a, h1b)

            def emit_z2_wave(h1p):
                """z2 wave: h1 chunk q at partitions 64*HH[q], out at 64*(q%2).
                Tiles (0,0),(64,64),(64,0),(0,64) -- all distinct, one wave."""
                h1a, h1b = h1p
                z2 = psz2.tile([128, 1024], F32, tag="z2")
                for q in range(4):
                    hp = 64 * HH[q]
                    gp = 64 * (q % 2)
                    cp = 512 * (q // 2)
                    src = h1a if q < 2 else h1b
                    nc.tensor.matmul(z2[gp:gp + 64, cp:cp + 512],
                                     wp1_t[hp:hp + 64, :],
                                     src[hp:hp + 64, :],
                                     start=True, stop=True, tile_position=(hp, gp))
                return z2

            def emit_reduce(sp, z2):
                """segment max; max(relu(z+b)) == relu(max(z)+b): relu+bias
                deferred to the pooled array."""
                src = z2[:].rearrange("p (d k) -> p d k", k=32)
                dst = pooled_raw[:, 32 * sp:32 * sp + 32].rearrange(
                    "p (d one) -> p d one", one=1)
                nc.vector.tensor_reduce(dst, src, op=MAX, axis=AX)

            # --- phase 3 (post-max MLP + output FC + residual + relu), staged
            # so each loop iteration carries at most one stage of work and the
            # steady pipeline is not disrupted.  The final tile is only 64
            # real pooled cols (rest is padding) and is shrunk accordingly.
            def p3_width(t):
                return 512

            def p3_stage0(ctx):
                t = ctx["t"]
                cw = min(512, PC - 512 * t)
                nc.scalar.activation(pooled[:, 512 * t:512 * t + cw],
                                     pooled_raw[:, 512 * t:512 * t + cw], RELU,
                                     bias=bp1_t, scale=1.0)

            def p3_stage1(ctx):
                t, cw = ctx["t"], ctx["cw"]
                c = 512 * t
                ps_p1 = psy.tile([128, 512], F32, tag="ps1", name=f"p1_{t}")
                nc.tensor.matmul(ps_p1[0:64, :cw], wq0_t[0:64, :],
                                 pooled[0:64, c:c + cw],
                                 start=True, stop=True, tile_position=(0, 0))
                nc.tensor.matmul(ps_p1[64:128, :cw], wq0_t[64:128, :],
                                 pooled[64:128, c:c + cw],
                                 start=True, stop=True, tile_position=(64, 64))
                p1 = ph3_p.tile([128, 512], F16, tag="p1")
                nc.scalar.activation(p1[:, :cw], ps_p1[:, :cw], RELU,
                                     bias=bq0_t, scale=1.0)
                ctx["p1"] = p1

            def p3_stage2(ctx):
                t, cw, p1 = ctx["t"], ctx["cw"], ctx["p1"]
                ps_p2 = psy.tile([128, 512], F32, tag="ps1", name=f"p2_{t}")
                nc.tensor.matmul(ps_p2[0:64, :cw], wq1_t[0:64, :], p1[0:64, :cw],
                                 start=True, stop=True, tile_position=(0, 0))
                nc.tensor.matmul(ps_p2[64:128, :cw], wq1_t[64:128, :],
                                 p1[64:128, :cw],
                                 start=True, stop=True, tile_position=(64, 64))
                p2 = ph3_p.tile([128, 512], F16, tag="p2")
                nc.scalar.activation(p2[:, :cw], ps_p2[:, :cw], RELU,
                                     bias=bq1_t, scale=1.0)
                ctx["p2"] = p2

            def p3_stage3(ctx):
                t, cw, p2 = ctx["t"], ctx["cw"], ctx["p2"]
                rf = psz.tile([128, 1024], F32, tag="z1", name=f"rf_{t}")
                nc.tensor.matmul(rf[:, 0:cw], wo_t[0:64, :], p2[0:64, :cw],
                                 start=True, stop=True, tile_position=(0, 0))
                nc.tensor.matmul(rf[:, cw:2 * cw], wo_t[64:128, :], p2[64:128, :cw],
                                 start=True, stop=True, tile_position=(64, 0))
                res_t = ph3_p.tile([128, 1024], BF16, tag="res")
                nc.sync.dma_start(res_t[:, :2 * cw],
                                  resid_bf[:, 1024 * t:1024 * t + 2 * cw])
                o32 = ph3_p.tile([128, 1024], F32, tag="o32")
                nc.vector.tensor_tensor(o32[:, :2 * cw], rf[:, :2 * cw],
                                        res_t[:, :2 * cw], op=ADD)
                ctx["o32"] = o32

            def p3_stage4(ctx):
                t, cw, o32 = ctx["t"], ctx["cw"], ctx["o32"]
                obf = ph3_p.tile([128, 1024], BF16, tag="obf")
                nc.vector.tensor_scalar_max(obf[:, :2 * cw], o32[:, :2 * cw], 0.0)
                nc.sync.dma_start(out_t[:, 1024 * t:1024 * t + 2 * cw],
                                  obf[:, :2 * cw])

            P3_STAGES = [p3_stage0, p3_stage1, p3_stage2, p3_stage3, p3_stage4]
            p3q = []   # pending (stage_idx, ctx)

            def p3_tick():
                if p3q:
                    si, ctx = p3q.pop(0)
                    P3_STAGES[si](ctx)
                    if si + 1 < len(P3_STAGES):
                        p3q.append((si + 1, ctx))

            # --- phase 2 main loop.  Per iteration the PE sees
            #   p(s), f1n(s+1), n(s), z2(s-1), c(s)  [+ staged phase-1/3 filler]
            # -- adjacent waves always target different PSUM tiles.
            cur = issue_group(0)
            f1n_cur = emit_f1n_relu(emit_f1n_wave(0, cur[0]))
            nxt = None
            h1_prev = None
            for s in range(S):
                if s % GRP == 0 and s + GRP < S:
                    nxt = issue_group(s // GRP + 1)
                z1 = emit_p_wave(s, cur[1])
                ps_y_next = None
                if s + 1 < S:
                    if (s + 1) % GRP == 0:
                        cur = nxt
                    ps_y_next = emit_f1n_wave(s + 1, cur[0])
                emit_n_wave(z1, f1n_cur)
                if h1_prev is not None:
                    z2 = emit_z2_wave(h1_prev)
                emit_c_wave(s, z1)
                # vector/scalar ops for this iteration
                if ps_y_next is not None:
                    f1n_cur = emit_f1n_relu(ps_y_next)
                if h1_prev is not None:
                    emit_reduce(s - 1, z2)
                h1_prev = emit_h1(z1)
                if s in (8, 40, 72):
                    emit_phase1({8: 1, 40: 2, 72: 3}[s])
                if s >= 1 and (s - 1) % 16 == 15:
                    p3q.append((0, {"t": (s - 1) // 16,
                                    "cw": p3_width((s - 1) // 16)}))
                p3_tick()
            z2 = emit_z2_wave(h1_prev)
            emit_reduce(S - 1, z2)
            while p3q:
                p3_tick()
            p3q.append((0, {"t": 6, "cw": p3_width(6)}))
            while p3q:
                p3_tick()

    nc.compile()
    return nc


def _dets_of_core(k):
    return np.arange(DC_REAL * k, DC_REAL * (k + 1))


def _host_prep(detFeatures, cIdxs, nIdxs, pairFeatures,
               W1, b1, Wp0, bp0, Wp1, bp1, Wq0, bq0, Wq1, bq1, Wo, bo):
    """Build per-core input maps. Returns (in_maps, out_col) where out_col
    maps device output columns back to det order."""
    f16 = np.float16
    f8 = mybir.dt.np(F8)
    bf = mybir.dt.np(BF16)
    detF = np.asarray(detFeatures, np.float32)
    pairF = np.asarray(pairFeatures, np.float32)
    nI = np.asarray(nIdxs, np.int64)

    # weights (shared across cores)
    W1_16 = np.ascontiguousarray(W1, np.float32).astype(f16)           # [128, 32]
    wn4 = np.tile(Wp0[64:96].astype(f16), (4, 1))                      # [128, 64]
    wc4 = np.tile(Wp0[32:64].astype(f16), (4, 1))                      # [128, 64]
    wp1_2 = np.tile(Wp1.astype(f16), (2, 1))                           # [128, 64]
    wq0_2 = np.tile(Wq0.astype(f16), (2, 1))
    wq1_2 = np.tile(Wq1.astype(f16), (2, 1))
    wo2 = np.tile(Wo.astype(f16), (2, 1))                              # [128, 128]
    W1_8 = np.ascontiguousarray(W1, np.float32).astype(f8)
    wp4_8 = np.ascontiguousarray(np.tile(Wp0[0:32], (4, 1))).astype(f8)
    b1x4 = np.tile(np.asarray(b1, np.float32), 4)[:, None]             # [128, 1]
    bp0x2 = np.tile(np.asarray(bp0, np.float32), 2)[:, None]
    bp1x2 = np.tile(np.asarray(bp1, np.float32), 2)[:, None]
    bq0x2 = np.tile(np.asarray(bq0, np.float32), 2)[:, None]
    bq1x2 = np.tile(np.asarray(bq1, np.float32), 2)[:, None]
    bo32 = np.asarray(bo, np.float32)

    # det-order scramble for pooled/output columns:
    # local det d: s = d//64, q = (d%64)//16, i = d%16
    d = np.arange(DC)
    s_, q_, i_ = d // 64, (d % 64) // 16, d % 16
    pooled_col = 32 * s_ + 16 * (q_ // 2) + i_
    half = q_ % 2
    t3 = pooled_col // 512
    out_col = 1024 * t3 + 512 * half + (pooled_col % 512)              # [DC]

    detF8 = detF.astype(f8)
    in_maps = []
    for k in range(N_CORES):
        dets = _dets_of_core(k)
        dloc = detF[dets]                                              # [6250, 128]
        dpad = np.zeros((DC, 128), np.float32)
        dpad[:DC_REAL] = dloc
        detft16 = np.ascontiguousarray(dpad.T.astype(f16))             # [128, DC]

        # resid_bf[:, out_col[d]] = detF[d] + bo  (scrambled; pads zero)
        resid = np.zeros((2 * PCP, 128), np.float32)
        resid[out_col[:DC_REAL]] = dloc + bo32
        resid_bf = np.ascontiguousarray(resid.T.astype(bf))            # [128, 2*PCP]

        # pairs of this core, padded
        pf = np.zeros((PAIRS, 32), np.float32)
        pf[:DC_REAL * KN] = pairF[DC_REAL * KN * k: DC_REAL * KN * (k + 1)]
        # strip packing: [S, 4, 512, 32] -> [4, 32, S, 512] -> [128, S*512]
        p8a = np.ascontiguousarray(
            pf.reshape(S, 4, 512, 32).transpose(1, 3, 0, 2).reshape(128, S * 512)
        ).astype(f8)

        ni = np.zeros(PAIRS, np.int64)
        ni[:DC_REAL * KN] = nI[DC_REAL * KN * k: DC_REAL * KN * (k + 1)]
        dfn8 = np.ascontiguousarray(detF8[ni].T)                       # [128, PAIRS]

        in_maps.append({
            "detft16": detft16, "p8": p8a, "dfn8": dfn8, "resid_bf": resid_bf,
            "w1": W1_16, "w1_8": W1_8, "wp4_8": wp4_8, "wn4": wn4, "wc4": wc4,
            "wp1_2": wp1_2, "wq0_2": wq0_2, "wq1_2": wq1_2, "wo2": wo2,
            "b1x4": b1x4, "bp0x2": bp0x2, "bp1x2": bp1x2,
            "bq0x2": bq0x2, "bq1x2": bq1x2,
        })
    return in_maps, out_col


def _run(inputs, trace=False, tmpdir=None):
    if "nc" not in _CACHE:
        _CACHE["nc"] = _build()
    nc = _CACHE["nc"]
    in_maps, out_col = _host_prep(**inputs)
    res = run_bass_kernel_spmd(nc, in_maps, core_ids=list(range(N_CORES)),
                               trace=trace, tmpdir=tmpdir)
    outs = []
    for k in range(N_CORES):
        ot = np.asarray(res.results[k]["out_t"])                       # [128, 2*PCP]
        outs.append(ot[:, out_col[:DC_REAL]].T)                        # [6250, 128]
    full = np.concatenate(outs, axis=0).astype(np.float32)
    return full, res


def kernel(**inputs):
    inputs = {k: np.asarray(v) for k, v in inputs.items()}
    full, _ = _run(inputs, trace=False)
    return full
